# revision 47
# baseline (speedup 1.0000x reference)
"""Trainium2 Bass kernel for nn_KuramotoHyperUniversal.

Data-parallel over batch across 8 NeuronCores (64 rows/core); weights
replicated in bf16 (halves HBM traffic vs f32; rel err ~1e-3 in numpy
simulation, well under the 2e-2 gate). The (B,D,D) pairwise term uses
  sum_j sin(y_j - y_i) A[i,j] = cos(y_i)*(S@A^T)_i - sin(y_i)*(C@A^T)_i
so it is two [64,512]x[512,512] matmuls instead of a 64MB tensor.

Layout strategy: matmuls keep activations transposed (feature-on-
partition) as the stationary operand and stream bf16 weights as rhs.
Input-side transposes (yd^T, freqs^T, A^T) are prepared on host; the
per-layer hidden transposes use the xbar DMA-transpose (bf16) issued
from the ACT engine's HWDGE ring so they don't head-of-line block the
weight DMA stream on the sync ring. Weights load as three 1.6MB DMAs
per layer (4 row-chunks each) for near line-rate HBM bandwidth. The
constant t-column of the MLP input is folded into the layer-0 bias.
"""

import numpy as np
from contextlib import ExitStack

import ml_dtypes

import concourse.bass as bass
import concourse.mybir as mybir
import concourse.tile as tile
from concourse.vector_clock import ScopedClock, VectorClock
from concourse.bass_utils import run_bass_kernel_spmd
from concourse.masks import make_identity

DIM = 512
BATCH = 512
NCORES = 8
BS = BATCH // NCORES  # 64
H = 2 + 3 * DIM  # 1538
IN_SZ = 1 + 3 * DIM  # 1537
F32 = mybir.dt.float32
BF16 = mybir.dt.bfloat16
FP8 = mybir.dt.float8e4  # ml_dtypes.float8_e4m3, max normal 240
# W1/W2 ship as fp8*W_SCALE (power of two, exact); folded back out via the
# tanh activation's scale parameter. W ~ N(0, 1/sqrt(1538)) so
# max|W|*1024 ~ 130 << 240; keeps small weights out of fp8 subnormals.
W_SCALE = 1024.0
PI_HALF = float(np.pi / 2.0)


def _split_drain_and_barrier(self, tick_clock, wait_clock):
    # Walrus in this container rejects >2 sync waits on one CTRL (drain)
    # instruction; emit one single-wait NOP per outstanding proc instead.
    gc = tick_clock.global_clock
    ticks = list(gc)
    for p, t in enumerate(ticks):
        if t > 0:
            v = [0] * len(ticks)
            v[p] = t
            nop = self.nc.sync.nop(nofuse=True, hint=f"drain_wait_{p}")
            wait_clock.add_sem_waits(nop.ins, ScopedClock({None: VectorClock(v)}))
    self.nc.sync.drain()
    self.nc.all_engine_barrier()
    popped = self.nc._tile_sem_poison_stack.pop()
    assert popped is self._sem_poison
    self.nc.clear_and_free_semaphores(list(self.sems.allocated().values()))
    self.nc.all_engine_barrier()


tile.TileContext._drain_and_barrier = _split_drain_and_barrier

_MAX_WAITS = 1


def _split_waits(nc, limit=_MAX_WAITS):
    """Walrus rejects instructions carrying more than `limit` sync waits;
    move the excess onto same-engine NOPs inserted just before."""
    import bass_rust

    n = 0
    for f in nc.m.functions:
        for bb in f.blocks:
            out = []
            for inst in bb.instructions:
                si = inst.sync_info
                if si is not None and si.on_wait and len(si.on_wait) > limit:
                    waits = list(si.on_wait)
                    extra, keep = waits[:-limit], waits[-limit:]
                    for i in range(0, len(extra), limit):
                        nop = mybir.InstNoOp(name=f"I-wsplit-{n}", engine=inst.engine)
                        n += 1
                        nop.sync_info = bass_rust.SyncInfo(
                            on_wait=extra[i : i + limit], on_update=[]
                        )
                        out.append(nop)
                    inst.sync_info = bass_rust.SyncInfo(
                        on_wait=keep, on_update=list(si.on_update)
                    )
                out.append(inst)
            bb.instructions = out


def _build(reps=1, split_waits=True):
    nc = bass.Bass()
    AF = mybir.ActivationFunctionType

    t_p = nc.declare_dram_parameter("t", [1], F32, isOutput=False)
    y_p = nc.declare_dram_parameter("y", [BS, DIM + 1], F32, isOutput=False)
    fr_p = nc.declare_dram_parameter("freqs", [BS, DIM], F32, isOutput=False)
    ydT_p = nc.declare_dram_parameter("ydT", [DIM, BS], F32, isOutput=False)
    frT_p = nc.declare_dram_parameter("freqsT", [DIM, BS], BF16, isOutput=False)
    AT_p = nc.declare_dram_parameter("AT", [DIM, DIM], BF16, isOutput=False)
    W_p = [
        nc.declare_dram_parameter("W0", [IN_SZ, H], BF16, isOutput=False),
        nc.declare_dram_parameter("W1", [H, H], FP8, isOutput=False),
        nc.declare_dram_parameter("W2", [H, H], FP8, isOutput=False),
        nc.declare_dram_parameter("W3", [H, DIM], BF16, isOutput=False),
    ]
    b0_p = nc.declare_dram_parameter("b0", [H], F32, isOutput=False)
    # bwN = [bias_row; W_N rows 1536:1538] packed on host (bf16, pre-scaled
    # by W_SCALE for N=1,2) — lets one K=3 matmul apply bias + K-tail.
    bw_p = [
        nc.declare_dram_parameter("bw1", [3, H], BF16, isOutput=False),
        nc.declare_dram_parameter("bw2", [3, H], BF16, isOutput=False),
        nc.declare_dram_parameter("bw3", [3, DIM], BF16, isOutput=False),
    ]
    out_p = nc.declare_dram_parameter("out", [BS, DIM + 1], F32, isOutput=True)

    with ExitStack() as ctx:
        tc = ctx.enter_context(tile.TileContext(nc))
        const = ctx.enter_context(tc.tile_pool(name="const", bufs=1))
        io = ctx.enter_context(tc.tile_pool(name="io", bufs=1))
        xtp = ctx.enter_context(tc.tile_pool(name="xtp", bufs=1))
        wp = ctx.enter_context(tc.tile_pool(name="wp", bufs=1))
        hp = ctx.enter_context(tc.tile_pool(name="hp", bufs=1))
        htp = ctx.enter_context(tc.tile_pool(name="htp", bufs=1))
        ps = ctx.enter_context(tc.tile_pool(name="ps", bufs=1, space="PSUM"))
        pst = ctx.enter_context(tc.tile_pool(name="pst", bufs=1, space="PSUM"))

        id64 = const.tile([64, 64], BF16, tag="id64")
        make_identity(nc, id64[:])
        ones = const.tile([1, BS], BF16, tag="ones")
        nc.vector.memset(ones[:], 1.0)
        pih = const.tile([128, 1], F32, tag="pih")
        nc.vector.memset(pih[:], PI_HALF)

        def _emit(rep):
            # ================= DMAs on the sync (SP) HWDGE ring =========
            t_sb = io.tile([1, 1], F32, tag="t")
            nc.sync.dma_start(out=t_sb[:], in_=t_p[None, :])
            y_sb = io.tile([BS, DIM], F32, tag="y")
            nc.sync.dma_start(out=y_sb[:], in_=y_p[:, 0:DIM])
            ydt = xtp.tile([128, 4, BS], F32, tag="ydt")
            nc.sync.dma_start(
                out=ydt[:], in_=ydT_p.rearrange("(c p) b -> p c b", p=128)
            )
            frT = xtp.tile([128, 4, BS], BF16, tag="frT")
            nc.sync.dma_start(
                out=frT[:], in_=frT_p.rearrange("(c p) b -> p c b", p=128)
            )
            # biases + misc small loads
            b0row = const.tile([1, H], F32, tag="b0row")
            nc.sync.dma_start(out=b0row[:], in_=b0_p[None, :])
            w0row = const.tile([1, H], BF16, tag="w0row")
            nc.sync.dma_start(out=w0row[:], in_=W_p[0][1024:1025, :])
            brow = []
            for l, (bp, n) in enumerate(zip(bw_p, [H, H, DIM])):
                bt = const.tile([3, n], BF16, tag=f"bw{l + 1}")
                nc.sync.dma_start(out=bt[:], in_=bp[:])
                brow.append(bt)
            fr = io.tile([BS, DIM], F32, tag="fr")
            nc.sync.dma_start(out=fr[:], in_=fr_p[:])

            # weight tiles: one big chunked DMA per row-slab (fewer, larger
            # transfers keep the HWDGE/SEQ occupancy low and DMA at line
            # rate). wNs[s] is [128, cs, n]; chunk k of the layer lives at
            # slab k//4 (W0) or one of the consolidated groups below.
            # Weight tiles: one big chunked DMA per group on the sync
            # HWDGE ring — fewer, larger transfers keep SEQ/HWDGE occupancy
            # low and DMA at line rate. W1/W2 are fp8(e4m3): the PE streams
            # e4m3 rhs at bf16 speed, halving their HBM traffic. (e3m4 and
            # SWDGE cast-in-flight were measured much slower; W0/W3 stay
            # bf16 for accuracy margin.)
            def load_grouped(Wl, n, groups, tag, dt):
                """groups: list of (row0, nchunks). Returns list of chunk
                APs [128, n] in layer-chunk order."""
                chunks = []
                for g, (r0, cs) in enumerate(groups):
                    wt = wp.tile([128, cs, n], dt, tag=f"{tag}g{g}")
                    nc.sync.dma_start(
                        out=wt[:],
                        in_=Wl[r0 : r0 + 128 * cs, :].rearrange(
                            "(c p) f -> p c f", p=128
                        ),
                    )
                    chunks.extend(wt[:, c, :] for c in range(cs))
                return chunks

            # W0 in 3 small groups so L0's first matmuls start ~4us sooner;
            # A^T lands between W0 and W1 (its trig matmuls run in the
            # L0->L1 boundary when the PE would idle anyway).
            w0s = load_grouped(W_p[0], H, ((0, 4), (512, 4), (1025, 4)), "w0", BF16)
            at = wp.tile([128, 4, DIM], BF16, tag="at")
            nc.sync.dma_start(
                out=at[:], in_=AT_p.rearrange("(c p) f -> p c f", p=128)
            )
            w1s = load_grouped(W_p[1], H, ((0, 12),), "w1", FP8)
            w2s = load_grouped(W_p[2], H, ((0, 12),), "w2", FP8)
            w3s = load_grouped(W_p[3], DIM, ((0, 12),), "w3", BF16)

            # ================= trig =====================================
            # HW Sin needs inputs in [-pi, pi]. Range-reduce with the
            # magic-number round trick (DVE has no mod/floor):
            #   k  = round(x/2pi)  via  (x/2pi + 1.5*2^23) - 1.5*2^23
            #   sin(x) = Sin(x - 2pi*k)
            #   cos(x) = Sin((x - 2pi*kc) + pi/2),  kc = round(x/2pi + 1/4)
            PI = float(np.pi)
            MAGIC = 12582912.0  # 1.5 * 2**23
            R2PI = 1.0 / (2 * PI)
            AOP = mybir.AluOpType

            def trig(src, part, pref, pool, dt):
                sh = list(src.shape)

                def reduce(vsrc, tag):
                    mk = pool.tile(sh, F32, tag=f"{tag}mk")
                    nc.vector.tensor_scalar(
                        mk[:], vsrc[:], MAGIC, -2 * PI, AOP.subtract, AOP.mult
                    )
                    ang = pool.tile(sh, F32, tag=f"{tag}ang")
                    nc.vector.tensor_add(ang[:], src[:], mk[:])
                    return ang

                v = pool.tile(sh, F32, tag=f"{pref}v")
                nc.scalar.activation(v[:], src[:], AF.Copy, scale=R2PI, bias=MAGIC)
                sin_t = pool.tile(sh, dt, tag=f"{pref}s")
                nc.scalar.activation(sin_t[:], reduce(v, f"{pref}s")[:], AF.Sin)

                w = pool.tile(sh, F32, tag=f"{pref}w")
                nc.scalar.activation(
                    w[:], src[:], AF.Copy, scale=R2PI, bias=0.25 + 0.0
                )
                v2 = pool.tile(sh, F32, tag=f"{pref}v2")
                nc.scalar.activation(v2[:], w[:], AF.Copy, bias=MAGIC)
                cos_t = pool.tile(sh, dt, tag=f"{pref}c")
                nc.scalar.activation(
                    cos_t[:], reduce(v2, f"{pref}c")[:], AF.Sin,
                    bias=pih[0:part, :],
                )
                return sin_t, cos_t

            # S^T, C^T bf16 [128,4,64] for matmul stationary operands
            stb, ctb = trig(ydt, 128, "xt", xtp, BF16)
            # natural C,S in f32 for the elementwise part
            S, C = trig(y_sb, BS, "n", io, F32)

            # ================= trig matmuls (PE) ========================
            # AS[b,i] = sum_j S[b,j] A[i,j] ; AC likewise with C. Emitted
            # at the L0->L1 boundary (PE waits on the h0 transposes there
            # anyway, and A^T has landed right after W0).
            def trig_matmuls():
                psAS = ps.tile([BS, DIM], F32, tag="trg0", name="psAS")
                psAC = ps.tile([BS, DIM], F32, tag="trg1", name="psAC")
                for c in range(4):
                    nc.tensor.matmul(
                        psAS[:], stb[:, c, :], at[:, c, :],
                        start=(c == 0), stop=(c == 3),
                    )
                for c in range(4):
                    nc.tensor.matmul(
                        psAC[:], ctb[:, c, :], at[:, c, :],
                        start=(c == 0), stop=(c == 3),
                    )
                # fs = C*AS - S*AC (DVE), f32
                fs = io.tile([BS, DIM], F32, tag="fs")
                nc.vector.tensor_mul(fs[:], C[:], psAS[:])
                fs2 = io.tile([BS, DIM], F32, tag="fs2")
                nc.vector.tensor_mul(fs2[:], S[:], psAC[:])
                nc.vector.tensor_sub(fs[:], fs[:], fs2[:])
                return fs

            # ================= bias prep (DVE) ==========================
            # b0' = b0 + (t-1) * W0[1024, :]
            tm1 = const.tile([1, 1], F32, tag="tm1")
            nc.vector.tensor_scalar_add(tm1[:], t_sb[:], -1.0)
            b0p_f = const.tile([1, H], F32, tag="b0p_f")
            nc.vector.tensor_scalar_mul(b0p_f[:], w0row[:], tm1[:])
            nc.vector.tensor_add(b0p_f[:], b0p_f[:], b0row[:])
            b0p = const.tile([1, H], BF16, tag="b0p")
            nc.vector.tensor_copy(b0p[:], b0p_f[:])

            # ================= MLP ======================================
            def mlp_layer(
                l, lhs_chunks, lhs_tail, w_chunks, rhs_tail, bias, act_fn,
                act_scale=1.0,
            ):
                """lhs_chunks: 12 APs [128,64] bf16; lhs_tail: [3,64] packed
                [ones; hT_tail] AP or None (L0 uses the plain ones row);
                w_chunks: 12 APs [128,n]; bias: [1,n] row (L0) or [3,n]
                packed [bias; W rows 1536:1538] (L1+). Returns h as 3x
                [BS,512] tiles + [BS,2] tail, or a [BS,512] f32 tile."""
                full = w_chunks[0].shape[1] == H
                n_sizes = [512, 512, 512, 2] if full else [512]
                psum = [
                    ps.tile([BS, n], F32, tag=f"ps{n_i}", name=f"ps{l}_{n_i}")
                    for n_i, n in enumerate(n_sizes)
                ]
                for k, xt in enumerate(lhs_chunks):
                    rhs = w_chunks[k]
                    off = 0
                    for n_i, n in enumerate(n_sizes):
                        nc.tensor.matmul(
                            psum[n_i][:], xt, rhs[:, off : off + n],
                            start=(k == 0), stop=False,
                        )
                        off += n
                # bias (+ K-tail rows when packed) in one matmul per bank
                lhs_b = ones[:] if lhs_tail is None else lhs_tail
                off = 0
                for n_i, n in enumerate(n_sizes):
                    nc.tensor.matmul(
                        psum[n_i][:], lhs_b, bias[:, off : off + n],
                        start=False, stop=True,
                    )
                    off += n
                if not full:
                    cf = io.tile([BS, DIM], F32, tag="cf")
                    nc.scalar.activation(cf[:], psum[0][:], act_fn, scale=act_scale)
                    return cf
                hs = []
                for n_i in range(3):
                    ht = hp.tile([BS, 512], BF16, tag=f"h{n_i}")
                    nc.scalar.activation(
                        ht[:], psum[n_i][:], act_fn, scale=act_scale
                    )
                    hs.append(ht)
                # hb = [ones | tanh(tail)] in one [64,3] tile, pre-packed for
                # the single PE transpose into next layer's bias+tail operand
                hb = hp.tile([BS, 3], BF16, tag="hb")
                nc.vector.memset(hb[:, 0:1], 1.0)
                nc.scalar.activation(
                    hb[:, 1:3], psum[3][:], act_fn, scale=act_scale
                )
                return hs, hb

            def transpose_h(hs, htl, l):
                """hs: 3 [64,512] bf16 tiles; htl [64,2]. Returns 12 chunk
                APs [128,64] + a packed [3,64] [ones; hT_tail] AP. The xbar
                DMA-transposes alternate between the SP and ACT HWDGE rings
                to split sequencer occupancy; both rings are past their
                serial work by the time these issue. The [64,2] tail goes
                PE-transpose + DVE copy into the packed bias operand."""
                chunks = []
                for s in range(3):
                    hT = htp.tile([128, 4, BS], BF16, tag=f"hT{s}")
                    for c in range(4):
                        eng = nc.sync if (c % 2 == 0) else nc.scalar
                        eng.dma_start(
                            out=hT[:, c, :],
                            in_=hs[s][:, c * 128 : (c + 1) * 128],
                            transpose=True,
                        )
                    chunks.extend(hT[:, c, :] for c in range(4))
                p2 = pst.tile([3, 64], BF16, tag="pstT")
                nc.tensor.transpose(p2[:], htl[:], id64[:])
                m3 = htp.tile([3, 64], BF16, tag="m3")
                nc.vector.tensor_copy(m3[:], p2[:])
                return chunks, m3[:]

            l0_chunks = (
                [ctb[:, c, :] for c in range(4)]
                + [stb[:, c, :] for c in range(4)]
                + [frT[:, c, :] for c in range(4)]
            )
            hs, htl = mlp_layer(0, l0_chunks, None, w0s, None, b0p, AF.Tanh)
            chunks, tail = transpose_h(hs, htl, 0)
            fs = trig_matmuls()
            # L1/L2 weights+biases arrive pre-scaled by W_SCALE (fp8);
            # the tanh's scale folds it back out.
            hs, htl = mlp_layer(
                1, chunks, tail, w1s, None, brow[0], AF.Tanh,
                act_scale=1.0 / W_SCALE,
            )
            chunks, tail = transpose_h(hs, htl, 1)
            hs, htl = mlp_layer(
                2, chunks, tail, w2s, None, brow[1], AF.Tanh,
                act_scale=1.0 / W_SCALE,
            )
            chunks, tail = transpose_h(hs, htl, 2)
            cf = mlp_layer(3, chunks, tail, w3s, None, brow[2], AF.Copy)

            # ================= outputs ==================================
            out_sb = io.tile([BS, DIM + 1], F32, tag="osb")
            fm = io.tile([BS, DIM], F32, tag="fm")
            nc.vector.tensor_mul(fm[:], cf[:], fs[:])
            nc.vector.tensor_scalar_mul(fm[:], fm[:], 1.0 / DIM)
            nc.vector.tensor_add(out_sb[:, 0:DIM], fm[:], fr[:])
            sq = io.tile([BS, DIM], F32, tag="sq")
            nc.scalar.activation(
                sq[:], cf[:], AF.Square, accum_out=out_sb[:, DIM : DIM + 1]
            )
            nc.sync.dma_start(out=out_p[:], in_=out_sb[:])

        for _rep in range(reps):
            _emit(_rep)

    if split_waits:
        _split_waits(nc)
    return nc


_NC_CACHE = {}


def prepare_in_maps(inputs):
    bf16 = ml_dtypes.bfloat16
    fp8 = ml_dtypes.float8_e4m3
    f32 = np.float32

    def wq(name):
        return np.ascontiguousarray(
            np.asarray(inputs[name], dtype=f32) * W_SCALE, dtype=fp8
        )

    shared = {
        "t": np.ascontiguousarray(inputs["t"], dtype=f32),
        "b0": np.ascontiguousarray(inputs["b0"], dtype=f32),
        "bw1": np.ascontiguousarray(
            np.concatenate(
                [
                    np.asarray(inputs["b1"], dtype=f32)[None, :],
                    np.asarray(inputs["W1"], dtype=f32)[1536:1538, :],
                ],
                axis=0,
            )
            * W_SCALE,
            dtype=bf16,
        ),
        "bw2": np.ascontiguousarray(
            np.concatenate(
                [
                    np.asarray(inputs["b2"], dtype=f32)[None, :],
                    np.asarray(inputs["W2"], dtype=f32)[1536:1538, :],
                ],
                axis=0,
            )
            * W_SCALE,
            dtype=bf16,
        ),
        "bw3": np.ascontiguousarray(
            np.concatenate(
                [
                    np.asarray(inputs["b3"], dtype=f32)[None, :],
                    np.asarray(inputs["W3"], dtype=f32)[1536:1538, :],
                ],
                axis=0,
            ),
            dtype=bf16,
        ),
        "W0": np.ascontiguousarray(inputs["W0"], dtype=bf16),
        "W1": wq("W1"),
        "W2": wq("W2"),
        "W3": np.ascontiguousarray(inputs["W3"], dtype=bf16),
        "AT": np.ascontiguousarray(np.asarray(inputs["A"], dtype=f32).T, dtype=bf16),
    }
    y = np.asarray(inputs["y"], dtype=f32)
    freqs = np.asarray(inputs["freqs"], dtype=f32)
    in_maps = []
    for i in range(NCORES):
        yi = y[i * BS : (i + 1) * BS]
        fi = freqs[i * BS : (i + 1) * BS]
        m = dict(shared)
        m["y"] = np.ascontiguousarray(yi)
        m["freqs"] = np.ascontiguousarray(fi)
        m["ydT"] = np.ascontiguousarray(yi[:, 0:DIM].T)
        m["freqsT"] = np.ascontiguousarray(fi.T, dtype=bf16)
        in_maps.append(m)
    return in_maps


def kernel(**inputs):
    key = "nc"
    if key not in _NC_CACHE:
        _NC_CACHE[key] = _build()
    nc = _NC_CACHE[key]

    in_maps = prepare_in_maps(inputs)
    res = run_bass_kernel_spmd(nc, in_maps, core_ids=list(range(NCORES)))
    out = np.concatenate([res.results[i]["out"] for i in range(NCORES)], axis=0)
    return out.astype(np.float32)


# revision 49
# speedup vs baseline: 1.3530x; 1.3530x over previous
"""Trainium2 Bass kernel for nn_KuramotoHyperUniversal.

Data-parallel over batch across 8 NeuronCores (64 rows/core); weights
replicated in bf16 (halves HBM traffic vs f32; rel err ~1e-3 in numpy
simulation, well under the 2e-2 gate). The (B,D,D) pairwise term uses
  sum_j sin(y_j - y_i) A[i,j] = cos(y_i)*(S@A^T)_i - sin(y_i)*(C@A^T)_i
so it is two [64,512]x[512,512] matmuls instead of a 64MB tensor.

Layout strategy: matmuls keep activations transposed (feature-on-
partition) as the stationary operand and stream bf16 weights as rhs.
Input-side transposes (yd^T, freqs^T, A^T) are prepared on host; the
per-layer hidden transposes use the xbar DMA-transpose (bf16) issued
from the ACT engine's HWDGE ring so they don't head-of-line block the
weight DMA stream on the sync ring. Weights load as three 1.6MB DMAs
per layer (4 row-chunks each) for near line-rate HBM bandwidth. The
constant t-column of the MLP input is folded into the layer-0 bias.
"""

import numpy as np
from contextlib import ExitStack

import ml_dtypes

import concourse.bass as bass
import concourse.mybir as mybir
import concourse.tile as tile
from concourse.vector_clock import ScopedClock, VectorClock
from concourse.bass_utils import run_bass_kernel_spmd
from concourse.masks import make_identity

DIM = 512
BATCH = 512
NCORES = 8
BS = BATCH // NCORES  # 64
H = 2 + 3 * DIM  # 1538
IN_SZ = 1 + 3 * DIM  # 1537
F32 = mybir.dt.float32
BF16 = mybir.dt.bfloat16
FP8 = mybir.dt.float8e4  # ml_dtypes.float8_e4m3, max normal 240
# W1/W2 ship as fp8*W_SCALE (power of two, exact); folded back out via the
# tanh activation's scale parameter. W ~ N(0, 1/sqrt(1538)) so
# max|W|*1024 ~ 130 << 240; keeps small weights out of fp8 subnormals.
W_SCALE = 1024.0
PI_HALF = float(np.pi / 2.0)


def _split_drain_and_barrier(self, tick_clock, wait_clock):
    # Walrus in this container rejects >2 sync waits on one CTRL (drain)
    # instruction; emit one single-wait NOP per outstanding proc instead.
    gc = tick_clock.global_clock
    ticks = list(gc)
    for p, t in enumerate(ticks):
        if t > 0:
            v = [0] * len(ticks)
            v[p] = t
            nop = self.nc.sync.nop(nofuse=True, hint=f"drain_wait_{p}")
            wait_clock.add_sem_waits(nop.ins, ScopedClock({None: VectorClock(v)}))
    self.nc.sync.drain()
    self.nc.all_engine_barrier()
    popped = self.nc._tile_sem_poison_stack.pop()
    assert popped is self._sem_poison
    self.nc.clear_and_free_semaphores(list(self.sems.allocated().values()))
    self.nc.all_engine_barrier()


tile.TileContext._drain_and_barrier = _split_drain_and_barrier

_MAX_WAITS = 1


def _split_waits(nc, limit=_MAX_WAITS):
    """Walrus rejects instructions carrying more than `limit` sync waits;
    move the excess onto same-engine NOPs inserted just before."""
    import bass_rust

    n = 0
    for f in nc.m.functions:
        for bb in f.blocks:
            out = []
            for inst in bb.instructions:
                si = inst.sync_info
                if si is not None and si.on_wait and len(si.on_wait) > limit:
                    waits = list(si.on_wait)
                    extra, keep = waits[:-limit], waits[-limit:]
                    for i in range(0, len(extra), limit):
                        nop = mybir.InstNoOp(name=f"I-wsplit-{n}", engine=inst.engine)
                        n += 1
                        nop.sync_info = bass_rust.SyncInfo(
                            on_wait=extra[i : i + limit], on_update=[]
                        )
                        out.append(nop)
                    inst.sync_info = bass_rust.SyncInfo(
                        on_wait=keep, on_update=list(si.on_update)
                    )
                out.append(inst)
            bb.instructions = out


def _build(reps=1, split_waits=True):
    nc = bass.Bass()
    AF = mybir.ActivationFunctionType

    t_p = nc.declare_dram_parameter("t", [1], F32, isOutput=False)
    y_p = nc.declare_dram_parameter("y", [BS, DIM + 1], F32, isOutput=False)
    fr_p = nc.declare_dram_parameter("freqs", [BS, DIM], F32, isOutput=False)
    ydT_p = nc.declare_dram_parameter("ydT", [DIM, BS], F32, isOutput=False)
    frT_p = nc.declare_dram_parameter("freqsT", [DIM, BS], BF16, isOutput=False)
    AT_p = nc.declare_dram_parameter("AT", [DIM, DIM], BF16, isOutput=False)
    W_p = [
        nc.declare_dram_parameter("W0", [IN_SZ, H], BF16, isOutput=False),
        nc.declare_dram_parameter("W1", [H, H], FP8, isOutput=False),
        nc.declare_dram_parameter("W2", [H, H], FP8, isOutput=False),
        nc.declare_dram_parameter("W3", [H, DIM], BF16, isOutput=False),
    ]
    b0_p = nc.declare_dram_parameter("b0", [H], F32, isOutput=False)
    # bwN = [bias_row; W_N rows 1536:1538] packed on host (bf16, pre-scaled
    # by W_SCALE for N=1,2) — lets one K=3 matmul apply bias + K-tail.
    bw_p = [
        nc.declare_dram_parameter("bw1", [3, H], BF16, isOutput=False),
        nc.declare_dram_parameter("bw2", [3, H], BF16, isOutput=False),
        nc.declare_dram_parameter("bw3", [3, DIM], BF16, isOutput=False),
    ]
    out_p = nc.declare_dram_parameter("out", [BS, DIM + 1], F32, isOutput=True)

    with ExitStack() as ctx:
        tc = ctx.enter_context(tile.TileContext(nc))
        const = ctx.enter_context(tc.tile_pool(name="const", bufs=1))
        io = ctx.enter_context(tc.tile_pool(name="io", bufs=1))
        xtp = ctx.enter_context(tc.tile_pool(name="xtp", bufs=1))
        wp = ctx.enter_context(tc.tile_pool(name="wp", bufs=1))
        hp = ctx.enter_context(tc.tile_pool(name="hp", bufs=2))
        htp = ctx.enter_context(tc.tile_pool(name="htp", bufs=2))
        ps = ctx.enter_context(tc.tile_pool(name="ps", bufs=1, space="PSUM"))
        pst = ctx.enter_context(tc.tile_pool(name="pst", bufs=1, space="PSUM"))

        id64 = const.tile([64, 64], BF16, tag="id64")
        make_identity(nc, id64[:])
        ones = const.tile([1, BS], BF16, tag="ones")
        nc.vector.memset(ones[:], 1.0)
        pih = const.tile([128, 1], F32, tag="pih")
        nc.vector.memset(pih[:], PI_HALF)

        def _emit(rep):
            # ================= DMAs on the sync (SP) HWDGE ring =========
            t_sb = io.tile([1, 1], F32, tag="t")
            nc.sync.dma_start(out=t_sb[:], in_=t_p[None, :])
            y_sb = io.tile([BS, DIM], F32, tag="y")
            nc.sync.dma_start(out=y_sb[:], in_=y_p[:, 0:DIM])
            ydt = xtp.tile([128, 4, BS], F32, tag="ydt")
            nc.sync.dma_start(
                out=ydt[:], in_=ydT_p.rearrange("(c p) b -> p c b", p=128)
            )
            frT = xtp.tile([128, 4, BS], BF16, tag="frT")
            nc.sync.dma_start(
                out=frT[:], in_=frT_p.rearrange("(c p) b -> p c b", p=128)
            )
            # biases + misc small loads
            b0row = const.tile([1, H], F32, tag="b0row")
            nc.sync.dma_start(out=b0row[:], in_=b0_p[None, :])
            w0row = const.tile([1, H], BF16, tag="w0row")
            nc.sync.dma_start(out=w0row[:], in_=W_p[0][1024:1025, :])
            brow = []
            for l, (bp, n) in enumerate(zip(bw_p, [H, H, DIM])):
                bt = const.tile([3, n], BF16, tag=f"bw{l + 1}")
                nc.sync.dma_start(out=bt[:], in_=bp[:])
                brow.append(bt)
            fr = io.tile([BS, DIM], F32, tag="fr")
            nc.sync.dma_start(out=fr[:], in_=fr_p[:])

            # weight tiles: one big chunked DMA per row-slab (fewer, larger
            # transfers keep the HWDGE/SEQ occupancy low and DMA at line
            # rate). wNs[s] is [128, cs, n]; chunk k of the layer lives at
            # slab k//4 (W0) or one of the consolidated groups below.
            # Weight tiles: one big chunked DMA per group on the sync
            # HWDGE ring — fewer, larger transfers keep SEQ/HWDGE occupancy
            # low and DMA at line rate. W1/W2 are fp8(e4m3): the PE streams
            # e4m3 rhs at bf16 speed, halving their HBM traffic. (e3m4 and
            # SWDGE cast-in-flight were measured much slower; W0/W3 stay
            # bf16 for accuracy margin.)
            def load_grouped(Wl, n, groups, tag, dt):
                """groups: list of (row0, nchunks). Returns list of chunk
                APs [128, n] in layer-chunk order."""
                chunks = []
                for g, (r0, cs) in enumerate(groups):
                    wt = wp.tile([128, cs, n], dt, tag=f"{tag}g{g}")
                    nc.sync.dma_start(
                        out=wt[:],
                        in_=Wl[r0 : r0 + 128 * cs, :].rearrange(
                            "(c p) f -> p c f", p=128
                        ),
                    )
                    chunks.extend(wt[:, c, :] for c in range(cs))
                return chunks

            # W0 in 3 small groups so L0's first matmuls start ~4us sooner;
            # A^T lands between W0 and W1 (its trig matmuls run in the
            # L0->L1 boundary when the PE would idle anyway).
            w0s = load_grouped(W_p[0], H, ((0, 4), (512, 4), (1025, 4)), "w0", BF16)
            at = wp.tile([128, 4, DIM], BF16, tag="at")
            nc.sync.dma_start(
                out=at[:], in_=AT_p.rearrange("(c p) f -> p c f", p=128)
            )
            w1s = load_grouped(W_p[1], H, ((0, 12),), "w1", FP8)
            w2s = load_grouped(W_p[2], H, ((0, 12),), "w2", FP8)
            w3s = load_grouped(W_p[3], DIM, ((0, 12),), "w3", BF16)

            # ================= trig =====================================
            # HW Sin needs inputs in [-pi, pi]. Range-reduce with the
            # magic-number round trick (DVE has no mod/floor):
            #   k  = round(x/2pi)  via  (x/2pi + 1.5*2^23) - 1.5*2^23
            #   sin(x) = Sin(x - 2pi*k)
            #   cos(x) = Sin((x - 2pi*kc) + pi/2),  kc = round(x/2pi + 1/4)
            PI = float(np.pi)
            MAGIC = 12582912.0  # 1.5 * 2**23
            R2PI = 1.0 / (2 * PI)
            AOP = mybir.AluOpType

            def trig(src, part, pref, pool, dt):
                sh = list(src.shape)

                def reduce(vsrc, tag):
                    mk = pool.tile(sh, F32, tag=f"{tag}mk")
                    nc.vector.tensor_scalar(
                        mk[:], vsrc[:], MAGIC, -2 * PI, AOP.subtract, AOP.mult
                    )
                    ang = pool.tile(sh, F32, tag=f"{tag}ang")
                    nc.vector.tensor_add(ang[:], src[:], mk[:])
                    return ang

                v = pool.tile(sh, F32, tag=f"{pref}v")
                nc.scalar.activation(v[:], src[:], AF.Copy, scale=R2PI, bias=MAGIC)
                sin_t = pool.tile(sh, dt, tag=f"{pref}s")
                nc.scalar.activation(sin_t[:], reduce(v, f"{pref}s")[:], AF.Sin)

                w = pool.tile(sh, F32, tag=f"{pref}w")
                nc.scalar.activation(
                    w[:], src[:], AF.Copy, scale=R2PI, bias=0.25 + 0.0
                )
                v2 = pool.tile(sh, F32, tag=f"{pref}v2")
                nc.scalar.activation(v2[:], w[:], AF.Copy, bias=MAGIC)
                cos_t = pool.tile(sh, dt, tag=f"{pref}c")
                nc.scalar.activation(
                    cos_t[:], reduce(v2, f"{pref}c")[:], AF.Sin,
                    bias=pih[0:part, :],
                )
                return sin_t, cos_t

            # S^T, C^T bf16 [128,4,64] for matmul stationary operands
            stb, ctb = trig(ydt, 128, "xt", xtp, BF16)
            # natural C,S in f32 for the elementwise part
            S, C = trig(y_sb, BS, "n", io, F32)

            # ================= trig matmuls (PE) ========================
            # AS[b,i] = sum_j S[b,j] A[i,j] ; AC likewise with C. Emitted
            # at the L0->L1 boundary (PE waits on the h0 transposes there
            # anyway, and A^T has landed right after W0).
            def trig_matmuls():
                psAS = ps.tile([BS, DIM], F32, tag="trg0", name="psAS")
                psAC = ps.tile([BS, DIM], F32, tag="trg1", name="psAC")
                for c in range(4):
                    nc.tensor.matmul(
                        psAS[:], stb[:, c, :], at[:, c, :],
                        start=(c == 0), stop=(c == 3),
                    )
                for c in range(4):
                    nc.tensor.matmul(
                        psAC[:], ctb[:, c, :], at[:, c, :],
                        start=(c == 0), stop=(c == 3),
                    )
                # fs = C*AS - S*AC (DVE), f32
                fs = io.tile([BS, DIM], F32, tag="fs")
                nc.vector.tensor_mul(fs[:], C[:], psAS[:])
                fs2 = io.tile([BS, DIM], F32, tag="fs2")
                nc.vector.tensor_mul(fs2[:], S[:], psAC[:])
                nc.vector.tensor_sub(fs[:], fs[:], fs2[:])
                return fs

            # ================= bias prep (DVE) ==========================
            # b0' = b0 + (t-1) * W0[1024, :]
            tm1 = const.tile([1, 1], F32, tag="tm1")
            nc.vector.tensor_scalar_add(tm1[:], t_sb[:], -1.0)
            b0p_f = const.tile([1, H], F32, tag="b0p_f")
            nc.vector.tensor_scalar_mul(b0p_f[:], w0row[:], tm1[:])
            nc.vector.tensor_add(b0p_f[:], b0p_f[:], b0row[:])
            b0p = const.tile([1, H], BF16, tag="b0p")
            nc.vector.tensor_copy(b0p[:], b0p_f[:])

            # ================= MLP ======================================
            def mlp_layer(
                l, lhs_chunks, lhs_tail, w_chunks, rhs_tail, bias, act_fn,
                act_scale=1.0,
            ):
                """lhs_chunks: 12 APs [128,64] bf16; lhs_tail: [3,64] packed
                [ones; hT_tail] AP or None (L0 uses the plain ones row);
                w_chunks: 12 APs [128,n]; bias: [1,n] row (L0) or [3,n]
                packed [bias; W rows 1536:1538] (L1+). Returns h as 3x
                [BS,512] tiles + [BS,2] tail, or a [BS,512] f32 tile."""
                full = w_chunks[0].shape[1] == H
                n_sizes = [512, 512, 512, 2] if full else [512]
                psum = [
                    ps.tile([BS, n], F32, tag=f"ps{n_i}", name=f"ps{l}_{n_i}")
                    for n_i, n in enumerate(n_sizes)
                ]
                for k, xt in enumerate(lhs_chunks):
                    rhs = w_chunks[k]
                    off = 0
                    for n_i, n in enumerate(n_sizes):
                        nc.tensor.matmul(
                            psum[n_i][:], xt, rhs[:, off : off + n],
                            start=(k == 0), stop=False,
                        )
                        off += n
                # bias (+ K-tail rows when packed) in one matmul per bank
                lhs_b = ones[:] if lhs_tail is None else lhs_tail
                off = 0
                for n_i, n in enumerate(n_sizes):
                    nc.tensor.matmul(
                        psum[n_i][:], lhs_b, bias[:, off : off + n],
                        start=False, stop=True,
                    )
                    off += n
                if not full:
                    cf = io.tile([BS, DIM], F32, tag="cf")
                    nc.scalar.activation(cf[:], psum[0][:], act_fn, scale=act_scale)
                    return cf
                hs = []
                for n_i in range(3):
                    ht = hp.tile([BS, 512], BF16, tag=f"h{n_i}")
                    nc.scalar.activation(
                        ht[:], psum[n_i][:], act_fn, scale=act_scale
                    )
                    hs.append(ht)
                # hb = [ones | tanh(tail)] in one [64,3] tile, pre-packed for
                # the single PE transpose into next layer's bias+tail operand
                hb = hp.tile([BS, 3], BF16, tag="hb")
                nc.vector.memset(hb[:, 0:1], 1.0)
                nc.scalar.activation(
                    hb[:, 1:3], psum[3][:], act_fn, scale=act_scale
                )
                return hs, hb

            def transpose_h(hs, htl, l):
                """hs: 3 [64,512] bf16 tiles; htl [64,2]. Returns 12 chunk
                APs [128,64] + a packed [3,64] [ones; hT_tail] AP. The xbar
                DMA-transposes alternate between the SP and ACT HWDGE rings
                to split sequencer occupancy; both rings are past their
                serial work by the time these issue. The [64,2] tail goes
                PE-transpose + DVE copy into the packed bias operand."""
                chunks = []
                for s in range(3):
                    hT = htp.tile([128, 4, BS], BF16, tag=f"hT{s}")
                    for c in range(4):
                        eng = nc.sync if (c % 2 == 0) else nc.scalar
                        eng.dma_start(
                            out=hT[:, c, :],
                            in_=hs[s][:, c * 128 : (c + 1) * 128],
                            transpose=True,
                        )
                    chunks.extend(hT[:, c, :] for c in range(4))
                p2 = pst.tile([3, 64], BF16, tag="pstT")
                nc.tensor.transpose(p2[:], htl[:], id64[:])
                m3 = htp.tile([3, 64], BF16, tag="m3")
                nc.vector.tensor_copy(m3[:], p2[:])
                return chunks, m3[:]

            l0_chunks = (
                [ctb[:, c, :] for c in range(4)]
                + [stb[:, c, :] for c in range(4)]
                + [frT[:, c, :] for c in range(4)]
            )
            hs, htl = mlp_layer(0, l0_chunks, None, w0s, None, b0p, AF.Tanh)
            chunks, tail = transpose_h(hs, htl, 0)
            fs = trig_matmuls()
            # L1/L2 weights+biases arrive pre-scaled by W_SCALE (fp8);
            # the tanh's scale folds it back out.
            hs, htl = mlp_layer(
                1, chunks, tail, w1s, None, brow[0], AF.Tanh,
                act_scale=1.0 / W_SCALE,
            )
            chunks, tail = transpose_h(hs, htl, 1)
            hs, htl = mlp_layer(
                2, chunks, tail, w2s, None, brow[1], AF.Tanh,
                act_scale=1.0 / W_SCALE,
            )
            chunks, tail = transpose_h(hs, htl, 2)
            cf = mlp_layer(3, chunks, tail, w3s, None, brow[2], AF.Copy)

            # ================= outputs ==================================
            out_sb = io.tile([BS, DIM + 1], F32, tag="osb")
            fm = io.tile([BS, DIM], F32, tag="fm")
            nc.vector.tensor_mul(fm[:], cf[:], fs[:])
            nc.vector.tensor_scalar_mul(fm[:], fm[:], 1.0 / DIM)
            nc.vector.tensor_add(out_sb[:, 0:DIM], fm[:], fr[:])
            sq = io.tile([BS, DIM], F32, tag="sq")
            nc.scalar.activation(
                sq[:], cf[:], AF.Square, accum_out=out_sb[:, DIM : DIM + 1]
            )
            nc.sync.dma_start(out=out_p[:], in_=out_sb[:])

        for _rep in range(reps):
            _emit(_rep)

    if split_waits:
        _split_waits(nc)
    return nc


_NC_CACHE = {}


def prepare_in_maps(inputs):
    bf16 = ml_dtypes.bfloat16
    fp8 = ml_dtypes.float8_e4m3
    f32 = np.float32

    def wq(name):
        return np.ascontiguousarray(
            np.asarray(inputs[name], dtype=f32) * W_SCALE, dtype=fp8
        )

    shared = {
        "t": np.ascontiguousarray(inputs["t"], dtype=f32),
        "b0": np.ascontiguousarray(inputs["b0"], dtype=f32),
        "bw1": np.ascontiguousarray(
            np.concatenate(
                [
                    np.asarray(inputs["b1"], dtype=f32)[None, :],
                    np.asarray(inputs["W1"], dtype=f32)[1536:1538, :],
                ],
                axis=0,
            )
            * W_SCALE,
            dtype=bf16,
        ),
        "bw2": np.ascontiguousarray(
            np.concatenate(
                [
                    np.asarray(inputs["b2"], dtype=f32)[None, :],
                    np.asarray(inputs["W2"], dtype=f32)[1536:1538, :],
                ],
                axis=0,
            )
            * W_SCALE,
            dtype=bf16,
        ),
        "bw3": np.ascontiguousarray(
            np.concatenate(
                [
                    np.asarray(inputs["b3"], dtype=f32)[None, :],
                    np.asarray(inputs["W3"], dtype=f32)[1536:1538, :],
                ],
                axis=0,
            ),
            dtype=bf16,
        ),
        "W0": np.ascontiguousarray(inputs["W0"], dtype=bf16),
        "W1": wq("W1"),
        "W2": wq("W2"),
        "W3": np.ascontiguousarray(inputs["W3"], dtype=bf16),
        "AT": np.ascontiguousarray(np.asarray(inputs["A"], dtype=f32).T, dtype=bf16),
    }
    y = np.asarray(inputs["y"], dtype=f32)
    freqs = np.asarray(inputs["freqs"], dtype=f32)
    in_maps = []
    for i in range(NCORES):
        yi = y[i * BS : (i + 1) * BS]
        fi = freqs[i * BS : (i + 1) * BS]
        m = dict(shared)
        m["y"] = np.ascontiguousarray(yi)
        m["freqs"] = np.ascontiguousarray(fi)
        m["ydT"] = np.ascontiguousarray(yi[:, 0:DIM].T)
        m["freqsT"] = np.ascontiguousarray(fi.T, dtype=bf16)
        in_maps.append(m)
    return in_maps


def kernel(**inputs):
    key = "nc"
    if key not in _NC_CACHE:
        _NC_CACHE[key] = _build()
    nc = _NC_CACHE[key]

    in_maps = prepare_in_maps(inputs)
    res = run_bass_kernel_spmd(nc, in_maps, core_ids=list(range(NCORES)))
    out = np.concatenate([res.results[i]["out"] for i in range(NCORES)], axis=0)
    return out.astype(np.float32)
